# revision 6
# baseline (speedup 1.0000x reference)
"""Multi-head self-attention (B=2, N=4096, D=512, h=8, d=64) on 8 TRN2 cores.

Sharding: batch*head-pair across the 8 cores (core c -> batch c//4, heads
2*(c%4), 2*(c%4)+1). Each core computes its two heads' q/k/v projections,
flash-style attention (scores kept transposed [j, i] so no P-matrix
transposes are ever needed; softmax denominators come from a ones-augmented
V stationary), and its partial output projection. Host sums the 4 partials
per batch and adds bo. No cross-core communication.

All matmuls run in bf16. q^T/k^T are stored d-replicated across both
partition halves so the d=64 score matmuls can be issued as row-tiled
pairs - tile_position (0,0) and (64,0) run concurrently in disjoint PE
row-groups, doubling score throughput vs K=128 zero-padded singles.
Scores land in 384-key psum tiles so each Exp activation covers N=1536
elements, amortizing the ACT engine's ~352-cycle per-instruction overhead
(the scalar engine is the overall bottleneck at ~33.5M exps per core).
"""

import numpy as np
import ml_dtypes

import concourse.bass as bass
import concourse.tile as tile
from concourse import bacc, mybir
from concourse.bass_utils import run_bass_kernel_spmd
from concourse.masks import make_identity

F32 = mybir.dt.float32
BF16 = mybir.dt.bfloat16

B, N, D = 2, 4096, 512
HEADS, DH = 8, 64
SCALE = DH ** -0.5          # 0.125
IC = 512                    # i-chunk (query cols per psum-out accumulation)
N_IC = N // IC              # 8
SUBS = N // 128             # 32 128-key sub-blocks per (ic, h)
N_TILES = 11                # 10 x 3-sub tiles + 1 x 2-sub tail tile
N_CORES = 8


def tile_subs(t):
    return 2 if t == N_TILES - 1 else 3


def build_kernel():
    nc = bacc.Bacc("TRN2", target_bir_lowering=False, debug=False)
    xT_d = nc.dram_tensor("xT", [D, N], BF16, kind="ExternalInput").ap()
    wq_d = nc.dram_tensor("wq", [D, 128], BF16, kind="ExternalInput").ap()
    wk_d = nc.dram_tensor("wk", [D, 128], BF16, kind="ExternalInput").ap()
    wv_d = nc.dram_tensor("wv", [D, 128], BF16, kind="ExternalInput").ap()
    wo_d = nc.dram_tensor("wo", [128, D], BF16, kind="ExternalInput").ap()
    pT_d = nc.dram_tensor("pT", [D, N], F32, kind="ExternalOutput").ap()
    # denominator-reciprocal scratch: one 512-row per (ic, head)
    dd1 = nc.dram_tensor("dscr1", [16, 512], F32).ap()
    dd2 = nc.dram_tensor("dscr2", [16, 512], F32).ap()

    with tile.TileContext(nc) as tc:
        with (
            tc.tile_pool(name="const", bufs=1) as const_pool,
            tc.tile_pool(name="proj", bufs=1) as proj_pool,
            tc.tile_pool(name="pt", bufs=3) as pt_pool,
            tc.tile_pool(name="norm", bufs=2) as norm_pool,
            tc.tile_pool(name="stage", bufs=3) as stage_pool,
            tc.tile_pool(name="ps", bufs=2, space="PSUM") as ps_pool,
            tc.tile_pool(name="po", bufs=2, space="PSUM") as po_pool,
        ):
            # ---- P0: loads + constants -------------------------------------
            w_sb = {}
            for nm, d_ap in (("wq", wq_d), ("wk", wk_d), ("wv", wv_d)):
                t = const_pool.tile([128, 4, 128], BF16, name=f"{nm}s", tag=f"{nm}s")
                nc.sync.dma_start(t[:], d_ap.rearrange("(c p) e -> p c e", p=128))
                w_sb[nm] = t
            wo_sb = const_pool.tile([128, D], BF16, name="wos", tag="wos")
            nc.sync.dma_start(wo_sb[:], wo_d[:])
            xt_sb = []
            for dc in range(4):
                t = const_pool.tile([128, N], BF16, name=f"xt{dc}", tag=f"xt{dc}")
                xt_sb.append(t)
            for i8 in range(8):
                for dc in range(4):
                    sl = slice(i8 * 512, (i8 + 1) * 512)
                    nc.sync.dma_start(xt_sb[dc][:, sl],
                                      xT_d[dc * 128:(dc + 1) * 128, sl])
            ident_f = const_pool.tile([128, 128], F32, name="ident_f",
                                      tag="ident_f")
            make_identity(nc, ident_f[:])
            ident = const_pool.tile([128, 128], BF16, name="ident", tag="ident")
            nc.vector.tensor_copy(ident[:], ident_f[:])
            # touch Exp once so the ACT table loads during the projection phase
            escr = const_pool.tile([1, 2], F32, name="escr", tag="escr")
            nc.scalar.activation(escr[:], ident_f[0:1, 0:2],
                                 mybir.ActivationFunctionType.Exp)

            # ---- P1: projections -------------------------------------------
            # Per-head q^T/k^T, d-replicated across both partition halves
            # (partitions 0..63 and 64..127 hold the same [d=64, n] data) so
            # score matmuls can run as concurrent row-tiled K=64 pairs.
            qTh = [proj_pool.tile([128, N], BF16, name=f"qTh{h}", tag=f"qTh{h}")
                   for h in range(2)]
            kTh = [proj_pool.tile([128, N], BF16, name=f"kTh{h}", tag=f"kTh{h}")
                   for h in range(2)]
            vT2 = proj_pool.tile([128, N], BF16, name="vT2", tag="vT2")
            # v natural [j, e] in bf16, ones-augmented per head (ones column
            # FIRST so the softmax denominator lands on psum partition 0):
            # v2aug[:, s, 0]=1, [1:65]=v_h0, [65]=1, [66:130]=v_h1
            v2aug = proj_pool.tile([128, SUBS, 130], BF16, name="v2aug",
                                   tag="v2aug")
            nc.gpsimd.memset(v2aug[:, :, 0:1], 1.0)
            nc.gpsimd.memset(v2aug[:, :, 65:66], 1.0)

            def proj_chunk_v(i8):
                sl = slice(i8 * 512, (i8 + 1) * 512)
                ps = ps_pool.tile([128, 512], F32, name="ps", tag="ps")
                for dc in range(4):
                    nc.tensor.matmul(
                        ps[:, 0:512],
                        w_sb["wv"][:, dc, :],
                        xt_sb[dc][:, sl],
                        start=(dc == 0),
                        stop=(dc == 3),
                    )
                nc.vector.tensor_copy(vT2[:, sl], ps[:, 0:512])

            def proj_chunk_qk(wname, i8, h, dst):
                # col-tiled pair: the same [128dc, 64] stationary is loaded at
                # array cols 0-63 and 64-127, both streaming the same x chunk
                # concurrently -> psum holds the head's [d=64, 512] replicated.
                sl = slice(i8 * 512, (i8 + 1) * 512)
                ps = ps_pool.tile([128, 512], F32, name="ps", tag="ps")
                for dc in range(4):
                    w_half = w_sb[wname][:, dc, h * 64:(h + 1) * 64]
                    nc.tensor.matmul(ps[0:64, :], w_half, xt_sb[dc][:, sl],
                                     start=(dc == 0), stop=(dc == 3))
                    nc.tensor.matmul(ps[64:128, :], w_half, xt_sb[dc][:, sl],
                                     start=(dc == 0), stop=(dc == 3))
                nc.vector.tensor_copy(dst[:, sl], ps[:, :])

            # ---- attention state machine per (ic, h) -----------------------
            # 128-key sub-blocks are issued as row-tiled concurrent pairs
            # (even sub in PE rows 0-63, odd sub in rows 64-127); scores for
            # 3 consecutive subs share a [128, 3, 512] psum tile = one
            # N=1536 Exp call. attn-out for tile t is emitted only after
            # tile t+1's scores, so the in-order PE queue never waits on exp.
            def make_att(ic, h):
                return {
                    "ic": ic, "h": h, "sc": {}, "pt": {}, "pending": None,
                    "pout": po_pool.tile([65, IC], F32, name="pout", tag="po"),
                }

            def att_attn_out(st, t):
                h = st["h"]
                pt = st["pt"][t]
                for ko in range(tile_subs(t)):
                    s = t * 3 + ko
                    nc.tensor.matmul(
                        st["pout"][:, :],
                        v2aug[:, s, h * 65:h * 65 + 65],
                        pt[:, ko, :],
                        start=(s == 0),
                        stop=(s == SUBS - 1),
                    )

            def att_pairs(st, pairs):
                ic, h = st["ic"], st["h"]
                isl = slice(ic * IC, (ic + 1) * IC)
                for p in pairs:
                    for s in (2 * p, 2 * p + 1):
                        t, ko = divmod(s, 3)
                        if t not in st["sc"]:
                            st["sc"][t] = ps_pool.tile(
                                [128, tile_subs(t), 512], F32,
                                name="sc", tag="ps")
                        half = slice((s % 2) * 64, (s % 2) * 64 + 64)
                        jsl = slice(s * 128, (s + 1) * 128)
                        nc.tensor.matmul(
                            st["sc"][t][:, ko, :],
                            kTh[h][half, jsl],
                            qTh[h][half, isl],
                            start=True,
                            stop=True,
                        )
                    for t in list(st["sc"]):
                        last_sub = t * 3 + tile_subs(t) - 1
                        if t not in st["pt"] and last_sub <= 2 * p + 1:
                            pt = pt_pool.tile([128, tile_subs(t), 512], BF16,
                                              name="pt", tag="pt")
                            nc.scalar.activation(
                                pt[:], st["sc"][t][:],
                                mybir.ActivationFunctionType.Exp, scale=SCALE)
                            st["pt"][t] = pt
                            if st["pending"] is not None:
                                att_attn_out(st, st["pending"])
                            st["pending"] = t

            def att_finish(st, outu):
                att_attn_out(st, st["pending"])
                ou = norm_pool.tile([65, IC], F32, name=f"outu{st['h']}",
                                    tag=f"outu{st['h']}")
                nc.vector.tensor_copy(ou[:], st["pout"][:])
                outu.append(ou)

            # q's first chunks feed the very first scores matmuls, then k/v
            # interleave chunk-wise (with v transposes) so attention can start
            # while the projection tail is still running.
            for i8 in range(2):
                proj_chunk_qk("wq", i8, 0, qTh[0])
                proj_chunk_qk("wq", i8, 1, qTh[1])
            st00 = None
            for i8 in range(8):
                proj_chunk_qk("wk", i8, 0, kTh[0])
                proj_chunk_qk("wk", i8, 1, kTh[1])
                proj_chunk_v(i8)
                if i8 >= 2:
                    proj_chunk_qk("wq", i8, 0, qTh[0])
                    proj_chunk_qk("wq", i8, 1, qTh[1])
                for s in range(4 * i8, 4 * i8 + 4):
                    psb = ps_pool.tile([128, 128], BF16, name="psb", tag="ps")
                    nc.tensor.transpose(
                        psb[:, 0:128], vT2[:, s * 128:(s + 1) * 128],
                        ident[:],
                    )
                    nc.vector.tensor_copy(v2aug[:, s, 1:65], psb[:, 0:64])
                    nc.vector.tensor_copy(v2aug[:, s, 66:130],
                                          psb[:, 64:128])
                # attention (ic0, h0) starts as soon as each key-block's
                # k/v tiles exist, overlapping the projection tail
                if st00 is None:
                    st00 = make_att(0, 0)
                att_pairs(st00, [2 * i8, 2 * i8 + 1])

            # ---- P2+P3: attention + normalize + output projection ----------
            norm_tiles = {}

            def norm_h(ic, h, outu):
                # Per-head normalize. The 512 distinct denominators take a
                # DRAM round-trip: row -> dram -> [128, 4] spread -> tiny DVE
                # reciprocal -> dram -> partition-broadcast DMA load. This
                # keeps multi-us reciprocals out of the in-order DVE stream.
                if h == 0:
                    norm_tiles[ic] = (
                        norm_pool.tile([128, IC], F32, name="st1", tag="st1"),
                        norm_pool.tile([128, IC], F32, name="rec", tag="rec"),
                        norm_pool.tile([128, IC], BF16, name="outn",
                                       tag="outn"),
                    )
                st1, rec, outn = norm_tiles[ic]
                psl = slice(h * 64, (h + 1) * 64)
                idx = ic * 2 + h
                spr = norm_pool.tile([128, 4], F32, name="spr", tag="spr")
                spro = norm_pool.tile([128, 4], F32, name="spro", tag="spro")
                nc.sync.dma_start(dd1[idx:idx + 1, :], outu[h][0:1, :])
                spread_ap = bass.AP(
                    tensor=dd1.tensor, offset=idx * 512,
                    ap=[[4, 128], [1, 4]],
                )
                nc.sync.dma_start(spr[:, :], spread_ap)
                nc.vector.reciprocal(spro[:, :], spr[:, :])
                spread_o = bass.AP(
                    tensor=dd2.tensor, offset=idx * 512,
                    ap=[[4, 128], [1, 4]],
                )
                nc.sync.dma_start(spread_o, spro[:, :])
                bcast_ap = bass.AP(
                    tensor=dd2.tensor, offset=idx * 512,
                    ap=[[0, 64], [1, 512]],
                )
                nc.sync.dma_start(rec[psl, :], bcast_ap)
                nc.sync.dma_start(st1[psl, :], outu[h][1:65, :])
                nc.vector.tensor_mul(outn[psl, :], st1[psl, :], rec[psl, :])
                return outn

            def p3_proj(ic, outn):
                # partial out projection: pT[oc, i] = wo[:, oc].T @ outn[:, i]
                for oc in range(4):
                    pp = ps_pool.tile([128, 512], F32, name="pp", tag="ps")
                    nc.tensor.matmul(
                        pp[:, 0:512],
                        wo_sb[:, oc * 128:(oc + 1) * 128],
                        outn[:, :],
                        start=True, stop=True,
                    )
                    st = stage_pool.tile([128, 512], F32, name="st",
                                         tag="st")
                    nc.vector.tensor_copy(st[:], pp[:, 0:512])
                    nc.sync.dma_start(
                        pT_d[oc * 128:(oc + 1) * 128,
                             ic * IC:(ic + 1) * IC],
                        st[:],
                    )

            # Software-pipelined emission, chosen so the in-order PE and
            # DVE instruction streams never wait on cross-engine chains:
            #   norm(ic-1,h1) before att(ic,h0); proj(ic-1) between the two
            #   head loops of att(ic); norm(ic,h0) before att(ic,h1).
            prev = None
            for ic in range(N_IC):
                if prev is not None:
                    norm_h(prev[0], 1, prev[1])
                outu = []
                if ic == 0:
                    att_finish(st00, outu)
                else:
                    st = make_att(ic, 0)
                    att_pairs(st, range(16))
                    att_finish(st, outu)
                if prev is not None:
                    p3_proj(prev[0], norm_tiles[prev[0]][2])
                norm_h(ic, 0, outu)
                st = make_att(ic, 1)
                att_pairs(st, range(16))
                att_finish(st, outu)
                prev = (ic, outu)
            norm_h(prev[0], 1, prev[1])
            p3_proj(prev[0], norm_tiles[prev[0]][2])
    nc.compile()
    return nc


_CACHE = {}


def _get_nc():
    if "nc" not in _CACHE:
        _CACHE["nc"] = build_kernel()
    return _CACHE["nc"]


def make_in_map(x, Wq, Wkv, Wo, core):
    bf = ml_dtypes.bfloat16
    b, p = divmod(core, 4)
    cs = slice(128 * p, 128 * (p + 1))
    return {
        "xT": np.ascontiguousarray(x[b].T).astype(bf),
        "wq": np.ascontiguousarray(Wq[:, cs]).astype(bf),
        "wk": np.ascontiguousarray(Wkv[:, :D][:, cs]).astype(bf),
        "wv": np.ascontiguousarray(Wkv[:, D:][:, cs]).astype(bf),
        "wo": np.ascontiguousarray(Wo[cs, :]).astype(bf),
    }


def kernel(x, Wq, Wkv, Wo, bo):
    x = np.asarray(x, dtype=np.float32)
    Wq = np.asarray(Wq, dtype=np.float32)
    Wkv = np.asarray(Wkv, dtype=np.float32)
    Wo = np.asarray(Wo, dtype=np.float32)
    bo = np.asarray(bo, dtype=np.float32)

    nc = _get_nc()
    in_maps = [make_in_map(x, Wq, Wkv, Wo, c) for c in range(N_CORES)]
    res = run_bass_kernel_spmd(nc, in_maps, core_ids=list(range(N_CORES)))
    out = np.empty((B, N, D), dtype=np.float32)
    for b in range(B):
        acc = res.results[4 * b]["pT"].copy()
        for p in range(1, 4):
            acc += res.results[4 * b + p]["pT"]
        out[b] = acc.T + bo
    return out


# revision 12
# speedup vs baseline: 1.0308x; 1.0308x over previous
"""Multi-head self-attention (B=2, N=4096, D=512, h=8, d=64) on 8 TRN2 cores.

Sharding: batch*head-pair across the 8 cores (core c -> batch c//4, heads
2*(c%4), 2*(c%4)+1). Each core computes its two heads' q/k/v projections,
flash-style attention (scores kept transposed [j, i] so no P-matrix
transposes are ever needed; softmax denominators come from a ones-augmented
V stationary), and its partial output projection. Host sums the 4 partials
per batch and adds bo. No cross-core communication.

All matmuls run in bf16. q^T/k^T are stored d-replicated across both
partition halves (replication via SBUF->SBUF DMA, off the engines) so the
d=64 score matmuls issue as row-tiled pairs - tile_position (0,0)/(64,0)
run concurrently in disjoint PE row-groups. Attention processes TWO
i-chunks per unit so every loaded stationary (k-block, v-block) feeds two
moving streams, halving exposed LDWEIGHTS cost. Scores land in 384-key
psum tiles so each Exp covers N=1536 elements, amortizing the ACT
engine's ~352-cycle per-instruction overhead (the scalar engine is the
overall bottleneck at ~33.5M exps per core).
"""

import numpy as np
import ml_dtypes

import concourse.bass as bass
import concourse.tile as tile
from concourse import bacc, mybir
from concourse.bass_utils import run_bass_kernel_spmd
from concourse.masks import make_identity

F32 = mybir.dt.float32
BF16 = mybir.dt.bfloat16

B, N, D = 2, 4096, 512
HEADS, DH = 8, 64
SCALE = DH ** -0.5          # 0.125
IC = 512                    # i-chunk (query cols per psum-out accumulation)
N_IC = N // IC              # 8
SUBS = N // 128             # 32 128-key sub-blocks per key sweep
N_TILES = 11                # 10 x 3-sub tiles + 1 x 2-sub tail tile
N_CORES = 8


def tile_subs(t):
    return 2 if t == N_TILES - 1 else 3


def build_kernel():
    nc = bacc.Bacc("TRN2", target_bir_lowering=False, debug=False)
    xT_d = nc.dram_tensor("xT", [D, N], BF16, kind="ExternalInput").ap()
    wq_d = nc.dram_tensor("wq", [D, 128], BF16, kind="ExternalInput").ap()
    wk_d = nc.dram_tensor("wk", [D, 128], BF16, kind="ExternalInput").ap()
    wv_d = nc.dram_tensor("wv", [D, 128], BF16, kind="ExternalInput").ap()
    wo_d = nc.dram_tensor("wo", [128, D], BF16, kind="ExternalInput").ap()
    pT_d = nc.dram_tensor("pT", [D, N], F32, kind="ExternalOutput").ap()
    # denominator-reciprocal scratch: one 512-row per (ic, head)
    dd1 = nc.dram_tensor("dscr1", [16, 512], F32).ap()
    dd2 = nc.dram_tensor("dscr2", [16, 512], F32).ap()

    with tile.TileContext(nc) as tc:
        with (
            tc.tile_pool(name="const", bufs=1) as const_pool,
            tc.tile_pool(name="proj", bufs=1) as proj_pool,
            tc.tile_pool(name="pt", bufs=4) as pt_pool,
            tc.tile_pool(name="norm", bufs=2) as norm_pool,
            tc.tile_pool(name="stage", bufs=3) as stage_pool,
            tc.tile_pool(name="ps", bufs=2, space="PSUM") as ps_pool,
            tc.tile_pool(name="po", bufs=2, space="PSUM") as po_pool,
        ):
            # ---- P0: loads + constants -------------------------------------
            w_sb = {}
            for nm, d_ap in (("wq", wq_d), ("wk", wk_d), ("wv", wv_d)):
                t = const_pool.tile([128, 4, 128], BF16, name=f"{nm}s", tag=f"{nm}s")
                nc.sync.dma_start(t[:], d_ap.rearrange("(c p) e -> p c e", p=128))
                w_sb[nm] = t
            wo_sb = const_pool.tile([128, D], BF16, name="wos", tag="wos")
            nc.sync.dma_start(wo_sb[:], wo_d[:])
            xt_sb = []
            for dc in range(4):
                t = const_pool.tile([128, N], BF16, name=f"xt{dc}", tag=f"xt{dc}")
                xt_sb.append(t)
            for i8 in range(8):
                for dc in range(4):
                    sl = slice(i8 * 512, (i8 + 1) * 512)
                    nc.sync.dma_start(xt_sb[dc][:, sl],
                                      xT_d[dc * 128:(dc + 1) * 128, sl])
            ident_f = const_pool.tile([128, 128], F32, name="ident_f",
                                      tag="ident_f")
            make_identity(nc, ident_f[:])
            ident = const_pool.tile([128, 128], BF16, name="ident", tag="ident")
            nc.vector.tensor_copy(ident[:], ident_f[:])
            # touch Exp once so the ACT table loads during the projection phase
            escr = const_pool.tile([1, 2], F32, name="escr", tag="escr")
            nc.scalar.activation(escr[:], ident_f[0:1, 0:2],
                                 mybir.ActivationFunctionType.Exp)

            # ---- P1: projections -------------------------------------------
            # Per-head q^T/k^T, d-replicated across both partition halves so
            # score matmuls can run as concurrent row-tiled K=64 pairs. The
            # projection itself is a single pass; replication happens via
            # SBUF->SBUF DMA (no engine cost).
            qTh = [proj_pool.tile([128, N], BF16, name=f"qTh{h}", tag=f"qTh{h}")
                   for h in range(2)]
            kTh = [proj_pool.tile([128, N], BF16, name=f"kTh{h}", tag=f"kTh{h}")
                   for h in range(2)]
            vT2 = proj_pool.tile([128, N], BF16, name="vT2", tag="vT2")
            # v natural [j, e] in bf16, ones-augmented per head (ones column
            # FIRST so the softmax denominator lands on psum partition 0):
            # v2aug[:, s, 0]=1, [1:65]=v_h0, [65]=1, [66:130]=v_h1
            v2aug = proj_pool.tile([128, SUBS, 130], BF16, name="v2aug",
                                   tag="v2aug")
            nc.gpsimd.memset(v2aug[:, :, 0:1], 1.0)
            nc.gpsimd.memset(v2aug[:, :, 65:66], 1.0)

            def proj_chunk(wname, i8, dsts):
                sl = slice(i8 * 512, (i8 + 1) * 512)
                ps = ps_pool.tile([128, 512], F32, name="ps", tag="ps")
                for dc in range(4):
                    nc.tensor.matmul(
                        ps[:, 0:512],
                        w_sb[wname][:, dc, :],
                        xt_sb[dc][:, sl],
                        start=(dc == 0),
                        stop=(dc == 3),
                    )
                if dsts is None:
                    nc.vector.tensor_copy(vT2[:, sl], ps[:, 0:512])
                else:
                    nc.vector.tensor_copy(dsts[0][0:64, sl], ps[0:64, 0:512])
                    nc.vector.tensor_copy(dsts[1][64:128, sl],
                                          ps[64:128, 0:512])
                    # d-replicate both heads via SBUF->SBUF DMA
                    nc.sync.dma_start(dsts[0][64:128, sl], dsts[0][0:64, sl])
                    nc.sync.dma_start(dsts[1][0:64, sl], dsts[1][64:128, sl])

            # ---- attention unit: two i-chunks x one head -------------------
            # 128-key sub-blocks issue as row-tiled concurrent pairs (even
            # sub in PE rows 0-63, odd sub in rows 64-127), each stationary
            # streaming both i-chunks. Scores for 3 consecutive subs share a
            # [128, 3, 512] psum tile per i-chunk = one N=1536 Exp call.
            # attn-out for tile t is emitted only after tile t+1's scores,
            # so the in-order PE queue never waits on exp.
            def make_att(icp, h, tsub=3):
                # tsub: subs per psum tile. The projection-overlapped unit
                # uses tsub=2 so tiles align with the 4-sub i8 chunks (a
                # tile never straddles a proj step - that would cycle psum
                # slot reuse through a future exp). Main units use tsub=3
                # (N=1536 Exp calls).
                nt = -(-SUBS // tsub)
                nsubs = [tsub] * nt
                if SUBS % tsub:
                    nsubs[-1] = SUBS % tsub
                return {
                    "icp": icp, "h": h, "sc": {}, "pt": {}, "pending": None,
                    "tsub": tsub, "nsubs": nsubs,
                    "pout": {ic: po_pool.tile([65, IC], F32, name="pout",
                                              tag="po") for ic in icp},
                }

            def att_attn_out(st, t):
                h = st["h"]
                for ko in range(st["nsubs"][t]):
                    s = t * st["tsub"] + ko
                    for ic in st["icp"]:
                        nc.tensor.matmul(
                            st["pout"][ic][:, :],
                            v2aug[:, s, h * 65:h * 65 + 65],
                            st["pt"][(t, ic)][:, ko, :],
                            start=(s == 0),
                            stop=(s == SUBS - 1),
                        )

            def att_pairs(st, pairs):
                # Emission is per 128-key sub-block: the even sub of each
                # pair targets PE rows 0-63, the odd sub rows 64-127, so
                # back-to-back subs run concurrently. A tile's Exp is
                # emitted the moment its last sub's matmuls are out - this
                # keeps psum-slot reuse acyclic when a pair spans two tiles
                # (the next tile's writes then simply wait on that exp).
                icp, h = st["icp"], st["h"]
                for p in pairs:
                    for s in (2 * p, 2 * p + 1):
                        t, ko = divmod(s, st["tsub"])
                        half = slice((s % 2) * 64, (s % 2) * 64 + 64)
                        jsl = slice(s * 128, (s + 1) * 128)
                        for ic in icp:
                            if (t, ic) not in st["sc"]:
                                st["sc"][(t, ic)] = ps_pool.tile(
                                    [128, st["nsubs"][t], 512], F32,
                                    name="sc", tag="ps")
                            nc.tensor.matmul(
                                st["sc"][(t, ic)][:, ko, :],
                                kTh[h][half, jsl],
                                qTh[h][half, ic * IC:(ic + 1) * IC],
                                start=True, stop=True,
                            )
                        if ko == st["nsubs"][t] - 1:
                            for ic in icp:
                                pt = pt_pool.tile(
                                    [128, st["nsubs"][t], 512], BF16,
                                    name="pt", tag="pt")
                                nc.scalar.activation(
                                    pt[:], st["sc"][(t, ic)][:],
                                    mybir.ActivationFunctionType.Exp,
                                    scale=SCALE)
                                st["pt"][(t, ic)] = pt
                            if st["pending"] is not None:
                                att_attn_out(st, st["pending"])
                            st["pending"] = t

            def att_finish(st, outu):
                att_attn_out(st, st["pending"])
                for ic in st["icp"]:
                    ou = norm_pool.tile([65, IC], F32,
                                        name=f"outu{st['h']}",
                                        tag=f"outu_{ic % 2}_{st['h']}")
                    nc.vector.tensor_copy(ou[:], st["pout"][ic][:])
                    outu[(ic, st["h"])] = ou

            # q's first chunks feed the very first scores matmuls, then k/v
            # interleave chunk-wise (with v transposes) so attention for
            # i-chunks 0+1 starts while the projection tail is still running.
            for i8 in range(2):
                proj_chunk("wq", i8, qTh)
            st00 = None
            outu_all = {}
            for i8 in range(8):
                proj_chunk("wk", i8, kTh)
                proj_chunk("wv", i8, None)
                if i8 >= 2:
                    proj_chunk("wq", i8, qTh)
                for s in range(4 * i8, 4 * i8 + 4):
                    psb = ps_pool.tile([128, 128], BF16, name="psb", tag="ps")
                    nc.tensor.transpose(
                        psb[:, 0:128], vT2[:, s * 128:(s + 1) * 128],
                        ident[:],
                    )
                    nc.vector.tensor_copy(v2aug[:, s, 1:65], psb[:, 0:64])
                    nc.vector.tensor_copy(v2aug[:, s, 66:130],
                                          psb[:, 64:128])
                if st00 is None:
                    st00 = make_att((0, 1), 0, tsub=2)
                att_pairs(st00, [2 * i8, 2 * i8 + 1])

            # ---- P2+P3: attention + normalize + output projection ----------
            norm_tiles = {}

            def norm_h(ic, h):
                # Per-head normalize. The 512 distinct denominators take a
                # DRAM round-trip: row -> dram -> [128, 4] spread -> tiny DVE
                # reciprocal -> dram -> partition-broadcast DMA load. This
                # keeps multi-us reciprocals out of the in-order DVE stream.
                ou = outu_all[(ic, h)]
                if h == 0:
                    norm_tiles[ic] = (
                        norm_pool.tile([128, IC], F32, name="st1",
                                       tag=f"st1_{ic % 2}"),
                        norm_pool.tile([128, IC], F32, name="rec",
                                       tag=f"rec_{ic % 2}"),
                        norm_pool.tile([128, IC], BF16, name="outn",
                                       tag=f"outn_{ic % 2}"),
                    )
                st1, rec, outn = norm_tiles[ic]
                psl = slice(h * 64, (h + 1) * 64)
                idx = ic * 2 + h
                spr = norm_pool.tile([128, 4], F32, name="spr", tag="spr")
                spro = norm_pool.tile([128, 4], F32, name="spro", tag="spro")
                nc.sync.dma_start(dd1[idx:idx + 1, :], ou[0:1, :])
                spread_ap = bass.AP(
                    tensor=dd1.tensor, offset=idx * 512,
                    ap=[[4, 128], [1, 4]],
                )
                nc.sync.dma_start(spr[:, :], spread_ap)
                nc.vector.reciprocal(spro[:, :], spr[:, :])
                spread_o = bass.AP(
                    tensor=dd2.tensor, offset=idx * 512,
                    ap=[[4, 128], [1, 4]],
                )
                nc.sync.dma_start(spread_o, spro[:, :])
                bcast_ap = bass.AP(
                    tensor=dd2.tensor, offset=idx * 512,
                    ap=[[0, 64], [1, 512]],
                )
                nc.sync.dma_start(rec[psl, :], bcast_ap)
                nc.sync.dma_start(st1[psl, :], ou[1:65, :])
                nc.vector.tensor_mul(outn[psl, :], st1[psl, :], rec[psl, :])

            def p3_proj(ic):
                # partial out projection: pT[oc, i] = wo[:, oc].T @ outn[:, i]
                outn = norm_tiles[ic][2]
                for oc in range(4):
                    pp = ps_pool.tile([128, 512], F32, name="pp", tag="ps")
                    nc.tensor.matmul(
                        pp[:, 0:512],
                        wo_sb[:, oc * 128:(oc + 1) * 128],
                        outn[:, :],
                        start=True, stop=True,
                    )
                    st = stage_pool.tile([128, 512], F32, name="st",
                                         tag="st")
                    nc.vector.tensor_copy(st[:], pp[:, 0:512])
                    nc.sync.dma_start(
                        pT_d[oc * 128:(oc + 1) * 128,
                             ic * IC:(ic + 1) * IC],
                        st[:],
                    )

            # Software-pipelined emission over i-chunk pairs:
            #   norm(prev,h1) before att(P,h0); p3(prev) between the two
            #   head units; norm(P,h0) before att(P,h1).
            ic_pairs = [(0, 1), (2, 3), (4, 5), (6, 7)]
            prevP = None
            for P in ic_pairs:
                if prevP is not None:
                    norm_h(prevP[0], 1)
                    norm_h(prevP[1], 1)
                if P == (0, 1):
                    att_finish(st00, outu_all)
                else:
                    st = make_att(P, 0)
                    att_pairs(st, range(16))
                    att_finish(st, outu_all)
                if prevP is not None:
                    p3_proj(prevP[0])
                    p3_proj(prevP[1])
                norm_h(P[0], 0)
                norm_h(P[1], 0)
                st = make_att(P, 1)
                att_pairs(st, range(16))
                att_finish(st, outu_all)
                prevP = P
            norm_h(prevP[0], 1)
            norm_h(prevP[1], 1)
            p3_proj(prevP[0])
            p3_proj(prevP[1])
    nc.compile()
    return nc


_CACHE = {}


def _get_nc():
    if "nc" not in _CACHE:
        _CACHE["nc"] = build_kernel()
    return _CACHE["nc"]


def make_in_map(x, Wq, Wkv, Wo, core):
    bf = ml_dtypes.bfloat16
    b, p = divmod(core, 4)
    cs = slice(128 * p, 128 * (p + 1))
    return {
        "xT": np.ascontiguousarray(x[b].T).astype(bf),
        "wq": np.ascontiguousarray(Wq[:, cs]).astype(bf),
        "wk": np.ascontiguousarray(Wkv[:, :D][:, cs]).astype(bf),
        "wv": np.ascontiguousarray(Wkv[:, D:][:, cs]).astype(bf),
        "wo": np.ascontiguousarray(Wo[cs, :]).astype(bf),
    }


def kernel(x, Wq, Wkv, Wo, bo):
    x = np.asarray(x, dtype=np.float32)
    Wq = np.asarray(Wq, dtype=np.float32)
    Wkv = np.asarray(Wkv, dtype=np.float32)
    Wo = np.asarray(Wo, dtype=np.float32)
    bo = np.asarray(bo, dtype=np.float32)

    nc = _get_nc()
    in_maps = [make_in_map(x, Wq, Wkv, Wo, c) for c in range(N_CORES)]
    res = run_bass_kernel_spmd(nc, in_maps, core_ids=list(range(N_CORES)))
    out = np.empty((B, N, D), dtype=np.float32)
    for b in range(B):
        acc = res.results[4 * b]["pT"].copy()
        for p in range(1, 4):
            acc += res.results[4 * b + p]["pT"]
        out[b] = acc.T + bo
    return out
